# revision 16
# baseline (speedup 1.0000x reference)
# Linear-attention layer (phi = elu+1) on 8 Trainium2 NeuronCores.
#
# Reference computation (per batch b):
#   q = x @ Wq + bq ; k = x @ Wk + bk ; v = x @ Wv + bv      [S, DM] each
#   kv[h] = phi(k_h)^T @ v_h          (sum over ALL of S)    [HD, HD]
#   attn_h = phi(q_h) @ kv[h]                                [S, HD]
#   out = attn @ Wo + bo                                     [S, DM]
#
# Sharding: sequence-parallel. Core c owns S/8 = 512 positions of every
# batch (2048 rows total). kv is a sum over sequence -> each core computes
# a partial kv over its rows, a 0.5 MiB AllReduce combines them, and every
# core finishes its own rows through attn + out_proj. Output rows are
# disjoint across cores, so no other communication is needed.
#
# phi(t) = elu(t) + 1 = exp(min(t, 0)) + relu(t)
#
# Numerics: x/W cast to bf16 on host, matmuls accumulate in fp32 PSUM,
# kv state reduced in bf16 (validated ~3.5e-3 scaled absmax vs fp32 ref).

import numpy as np
import ml_dtypes

B, S, DM, H = 4, 4096, 1024, 16
HD = DM // H          # 64
N_CORES = 8
P = 128
SC = S // N_CORES     # 512 sequence positions per core
R = B * SC            # 2048 rows per core
KC = DM // P          # 8 contraction chunks
NB = SC               # rows per batch on a core (512)
SCB = NB // P         # s-chunks per batch (4)
SCH = R // P          # s-chunks total (16)
NT_R = R // 512       # 512-wide row tiles (4)
ND = DM // 512        # 512-wide feature tiles (2)

_cache = {}


def _build(has_bias):
    import concourse.mybir as mybir
    import concourse.tile as tile
    from concourse import bacc

    fp32 = mybir.dt.float32
    bf16 = mybir.dt.bfloat16
    AF = mybir.ActivationFunctionType
    ALU = mybir.AluOpType

    nc = bacc.Bacc("TRN2", target_bir_lowering=False, debug=False,
                   num_devices=N_CORES)

    x_d = nc.dram_tensor("x", [R, DM], bf16, kind="ExternalInput").ap()
    wq_d = nc.dram_tensor("wq", [DM, DM], bf16, kind="ExternalInput").ap()
    wk_d = nc.dram_tensor("wk", [DM, DM], bf16, kind="ExternalInput").ap()
    wv_d = nc.dram_tensor("wv", [DM, DM], bf16, kind="ExternalInput").ap()
    wo_d = nc.dram_tensor("wo", [DM, DM], bf16, kind="ExternalInput").ap()
    bqc_d = nc.dram_tensor("bqc", [P, KC], fp32, kind="ExternalInput").ap()
    if has_bias:
        bk_d = nc.dram_tensor("bk2", [1, DM], bf16, kind="ExternalInput").ap()
        bv_d = nc.dram_tensor("bv2", [1, DM], bf16, kind="ExternalInput").ap()
        bo_d = nc.dram_tensor("bo2", [1, DM], bf16, kind="ExternalInput").ap()
    out_d = nc.dram_tensor("out", [R, DM], fp32, kind="ExternalOutput").ap()
    if _cache.get("debug"):
        dkv_d = nc.dram_tensor("dkv", [P, B * (H // 2) * HD], fp32,
                               kind="ExternalOutput").ap()
        dkvr_d = nc.dram_tensor("dkvr", [P, B * (H // 2) * HD], fp32,
                                kind="ExternalOutput").ap()
        dphiq_d = nc.dram_tensor("dphiq", [P, KC * R], fp32,
                                 kind="ExternalOutput").ap()
        dattnt_d = nc.dram_tensor("dattnt", [P, KC * R], fp32,
                                  kind="ExternalOutput").ap()
        dxt_d = nc.dram_tensor("dxt", [P, KC * R], fp32,
                               kind="ExternalOutput").ap()

    with tile.TileContext(nc) as tc:
        with (
            tc.tile_pool(name="big", bufs=1) as big,
            tc.tile_pool(name="stream", bufs=3) as stream,
            tc.tile_pool(name="tmp", bufs=2) as tmpp,
            tc.tile_pool(name="outp", bufs=3) as outp,
            tc.tile_pool(name="psum", bufs=6, space="PSUM") as psum,
            tc.tile_pool(name="kvps", bufs=1, space="PSUM") as kvps,
            tc.tile_pool(name="dram", bufs=1, space="DRAM") as dram,
        ):
            # ---------------- persistent tiles ----------------
            # wk/wv live only through phase 1 (own pool, closed after);
            # attnt is allocated afterwards and reuses their space.
            wkv_pool = tc.tile_pool(name="wkv", bufs=1)
            wkv = wkv_pool.__enter__()
            xt = big.tile([P, KC, R], bf16, tag="xt")        # x^T
            wq = big.tile([P, KC, DM], bf16, tag="wq")
            wk = wkv.tile([P, KC, DM], bf16, tag="wk")
            wv = wkv.tile([P, KC, DM], bf16, tag="wv")
            wo = big.tile([P, KC, DM], bf16, tag="wo")
            phiq = big.tile([P, KC, R], bf16, tag="phiq")    # phi(q)^T
            # kv state: head-pair stacked on partitions (even head rows 0:64,
            # odd head rows 64:128); column slot (b*8 + pair)*64
            kv_sb = big.tile([P, B * (H // 2) * HD], bf16, tag="kv")
            kv_rd = big.tile([P, B * (H // 2) * HD], bf16, tag="kvr")
            bqc = big.tile([P, KC], fp32, tag="bqc")
            if has_bias:
                bk2 = big.tile([1, DM], bf16, tag="bk2")
                bv2 = big.tile([1, DM], bf16, tag="bv2")
                bo2 = big.tile([1, DM], bf16, tag="bo2")
            ones = big.tile([1, P], bf16, tag="ones")
            zrow = big.tile([1, 512], bf16, tag="zrow")

            kv_in = dram.tile([P, B * (H // 2) * HD], bf16)
            kv_out = dram.tile([P, B * (H // 2) * HD], bf16)

            def s512(n):
                return slice(n * 512, (n + 1) * 512)

            # ---------------- loads ----------------
            # x transposes on the sync HWDGE queue, row-group split so the
            # first k/v projection chunks unblock early; weights on the
            # scalar HWDGE queue (parallel), in phase order (wk/wv first).
            for rg in range(NT_R):
                for d in range(KC):
                    nc.sync.dma_start(
                        xt[:, d, s512(rg)],
                        x_d[rg * 512:(rg + 1) * 512, d * P:(d + 1) * P],
                        transpose=True)
            for w_sb, w_dr in ((wk, wk_d), (wv, wv_d), (wq, wq_d), (wo, wo_d)):
                for c in range(KC):
                    nc.scalar.dma_start(
                        w_sb[:, c, :],
                        w_dr[c * P:(c + 1) * P, :])
            nc.gpsimd.dma_start(bqc[:], bqc_d)
            if has_bias:
                nc.gpsimd.dma_start(bk2[:], bk_d)
                nc.gpsimd.dma_start(bv2[:], bv_d)
                nc.gpsimd.dma_start(bo2[:], bo_d)
            nc.gpsimd.memset(ones[:], 1.0)
            nc.gpsimd.memset(zrow[:], 0.0)

            # ---------- phase 1: k/v projections + phi(k) + partial kv ----------
            for b in range(B):
                kvp = [kvps.tile([P, 512], fp32, tag=f"kvp{j}", name=f"kvp{j}")
                       for j in (0, 1)]
                for j in (0, 1):
                    # start=True zeroes the whole PSUM bank; do it exactly once
                    # per bank (full-width) so the per-head slot matmuls below
                    # can all accumulate with start=False.
                    nc.tensor.matmul(kvp[j][:], lhsT=ones[:], rhs=zrow[:],
                                     start=True, stop=False)
                for sc in range(SCB):
                    g = b * SCB + sc
                    kch = stream.tile([P, DM], bf16, tag="kch")
                    vch = stream.tile([P, DM], bf16, tag="vch")
                    for n in range(ND):
                        kps = psum.tile([P, 512], fp32, tag="pp")
                        vps = psum.tile([P, 512], fp32, tag="pp")
                        for kc in range(KC):
                            nc.tensor.matmul(
                                kps[:], lhsT=xt[:, kc, g * P:(g + 1) * P],
                                rhs=wk[:, kc, s512(n)],
                                start=(kc == 0),
                                stop=(not has_bias and kc == KC - 1))
                        if has_bias:
                            nc.tensor.matmul(kps[:], lhsT=ones[:],
                                             rhs=bk2[:, s512(n)],
                                             start=False, stop=True)
                        for kc in range(KC):
                            nc.tensor.matmul(
                                vps[:], lhsT=xt[:, kc, g * P:(g + 1) * P],
                                rhs=wv[:, kc, s512(n)],
                                start=(kc == 0),
                                stop=(not has_bias and kc == KC - 1))
                        if has_bias:
                            nc.tensor.matmul(vps[:], lhsT=ones[:],
                                             rhs=bv2[:, s512(n)],
                                             start=False, stop=True)
                        # phi(k) = exp(min(k,0)) + relu(k)
                        ut = tmpp.tile([P, 512], fp32, tag="u")
                        nc.vector.tensor_scalar_min(out=ut[:], in0=kps[:],
                                                    scalar1=0.0)
                        et = tmpp.tile([P, 512], bf16, tag="e")
                        nc.scalar.activation(out=et[:], in_=ut[:], func=AF.Exp)
                        rt = tmpp.tile([P, 512], bf16, tag="r")
                        nc.scalar.activation(out=rt[:], in_=kps[:], func=AF.Relu)
                        nc.vector.tensor_add(out=kch[:, s512(n)], in0=et[:],
                                             in1=rt[:])
                        nc.scalar.activation(out=vch[:, s512(n)], in_=vps[:],
                                             func=AF.Copy)
                    for h in range(H):
                        j, col = h // 8, (h % 8) * HD
                        # stationary = full head-pair block of phi(k); the
                        # off-pair half of the output is unused cross-terms
                        nc.tensor.matmul(
                            kvp[j][:, col:col + HD],
                            lhsT=kch[:, (h // 2) * P:(h // 2 + 1) * P],
                            rhs=vch[:, h * HD:(h + 1) * HD],
                            start=False,
                            stop=(sc == SCB - 1 and h % 8 == 7),
                            skip_group_check=True)
                for h in range(H):
                    j, col = h // 8, (h % 8) * HD
                    rows = slice((h % 2) * HD, (h % 2 + 1) * HD)
                    slot = (b * (H // 2) + h // 2) * HD
                    nc.scalar.activation(
                        out=kv_sb[rows, slot:slot + HD],
                        in_=kvp[j][rows, col:col + HD], func=AF.Copy)

            # wk/wv dead from here; free their SBUF for attnt
            wkv_pool.__exit__(None, None, None)
            attnt = big.tile([P, KC, R], bf16, tag="attnt")  # attn^T

            # ---------- phase 2: kv AllReduce (runs on TOPSP, overlaps phase 3) ----------
            nc.gpsimd.dma_start(kv_in[:], kv_sb[:])
            nc.gpsimd.collective_compute(
                "AllReduce",
                mybir.AluOpType.add,
                replica_groups=[list(range(N_CORES))],
                ins=[kv_in.opt()],
                outs=[kv_out.opt()],
            )
            nc.gpsimd.dma_start(kv_rd[:], kv_out[:])

            # ---------- phase 3: q^T projection + phi ----------
            for m in range(KC):
                for nt in range(NT_R):
                    qps = psum.tile([P, 512], fp32, tag="pp")
                    for kc in range(KC):
                        nc.tensor.matmul(
                            qps[:], lhsT=wq[:, kc, m * P:(m + 1) * P],
                            rhs=xt[:, kc, s512(nt)],
                            start=(kc == 0), stop=(kc == KC - 1))
                    ut = tmpp.tile([P, 512], fp32, tag="u")
                    nc.vector.tensor_scalar(out=ut[:], in0=qps[:],
                                            scalar1=bqc[:, m:m + 1],
                                            scalar2=0.0,
                                            op0=ALU.add, op1=ALU.min)
                    et = tmpp.tile([P, 512], bf16, tag="e")
                    nc.scalar.activation(out=et[:], in_=ut[:], func=AF.Exp)
                    rt = tmpp.tile([P, 512], bf16, tag="r")
                    nc.scalar.activation(out=rt[:], in_=qps[:], func=AF.Relu,
                                         bias=bqc[:, m:m + 1], scale=1.0)
                    nc.vector.tensor_add(out=phiq[:, m, s512(nt)], in0=et[:],
                                         in1=rt[:])

            # ---------- phase 4: attn^T = kv^T @ phi(q)^T per (b, head pair) ----------
            for b in range(B):
                for pr in range(H // 2):
                    ap = psum.tile([P, NB], fp32, tag="pp")
                    slot = (b * (H // 2) + pr) * HD
                    for half in range(2):
                        rows = slice(half * HD, (half + 1) * HD)
                        nc.tensor.matmul(
                            ap[rows, :],
                            lhsT=kv_rd[rows, slot:slot + HD],
                            rhs=phiq[rows, pr, b * NB:(b + 1) * NB],
                            start=True, stop=True)
                    nc.scalar.activation(
                        out=attnt[:, pr, b * NB:(b + 1) * NB],
                        in_=ap[:], func=AF.Copy)

            # ---------- phase 5: out = attn @ Wo + bo ----------
            for g in range(SCH):
                for n in range(ND):
                    ops = psum.tile([P, 512], fp32, tag="pp")
                    for kc in range(KC):
                        nc.tensor.matmul(
                            ops[:], lhsT=attnt[:, kc, g * P:(g + 1) * P],
                            rhs=wo[:, kc, s512(n)],
                            start=(kc == 0),
                            stop=(not has_bias and kc == KC - 1))
                    if has_bias:
                        nc.tensor.matmul(ops[:], lhsT=ones[:],
                                         rhs=bo2[:, s512(n)],
                                         start=False, stop=True)
                    osb = outp.tile([P, 512], fp32, tag="osb")
                    nc.scalar.activation(out=osb[:], in_=ops[:], func=AF.Copy)
                    nc.sync.dma_start(out_d[g * P:(g + 1) * P, s512(n)], osb[:])

            if _cache.get("debug"):
                for src_t, dst in ((kv_sb, dkv_d), (kv_rd, dkvr_d)):
                    for n in range(4):
                        f32t = outp.tile([P, 512], fp32, tag="osb")
                        nc.vector.tensor_copy(out=f32t[:], in_=src_t[:, s512(n)])
                        nc.sync.dma_start(dst[:, s512(n)], f32t[:])
                for src_t, dst in ((phiq, dphiq_d), (attnt, dattnt_d), (xt, dxt_d)):
                    for m in range(KC):
                        for n in range(NT_R):
                            f32t = outp.tile([P, 512], fp32, tag="osb")
                            nc.vector.tensor_copy(out=f32t[:],
                                                  in_=src_t[:, m, s512(n)])
                            nc.sync.dma_start(dst[:, m * R + n * 512:
                                                  m * R + (n + 1) * 512], f32t[:])

    nc.compile()
    return nc


def _get_nc(has_bias):
    key = ("nc", has_bias)
    if key not in _cache:
        _cache[key] = _build(has_bias)
    return _cache[key]


def _has_bias(inputs):
    return any(np.any(np.asarray(inputs[k], np.float32))
               for k in ("bk", "bv", "bo"))


def _make_in_maps(inputs, has_bias):
    bf16 = ml_dtypes.bfloat16
    x = np.asarray(inputs["x"], dtype=np.float32)
    ws = {k: np.ascontiguousarray(np.asarray(inputs[k], np.float32).astype(bf16))
          for k in ("Wq", "Wk", "Wv", "Wo")}
    bq = np.asarray(inputs["bq"], np.float32)
    bqc = np.ascontiguousarray(bq.reshape(KC, P).T.astype(np.float32))
    brow = {k: np.ascontiguousarray(
                np.asarray(inputs[k], np.float32).astype(bf16).reshape(1, DM))
            for k in ("bk", "bv", "bo")}
    xb = x.astype(bf16)
    in_maps = []
    for c in range(N_CORES):
        xs = np.ascontiguousarray(
            xb[:, c * SC:(c + 1) * SC, :].reshape(R, DM))
        m = {
            "x": xs,
            "wq": ws["Wq"], "wk": ws["Wk"], "wv": ws["Wv"], "wo": ws["Wo"],
            "bqc": bqc,
        }
        if has_bias:
            m.update({"bk2": brow["bk"], "bv2": brow["bv"],
                      "bo2": brow["bo"]})
        in_maps.append(m)
    return in_maps


def _run(inputs, **kw):
    from concourse import bass_utils
    hb = _has_bias(inputs)
    nc = _get_nc(hb)
    in_maps = _make_in_maps(inputs, hb)
    res = bass_utils.run_bass_kernel_spmd(
        nc, in_maps, core_ids=list(range(N_CORES)), **kw)
    out = np.empty((B, S, DM), np.float32)
    for c in range(N_CORES):
        out[:, c * SC:(c + 1) * SC, :] = res.results[c]["out"].reshape(B, SC, DM)
    return out, res


def kernel(**inputs) -> np.ndarray:
    out, _ = _run(inputs)
    return out


# revision 17
# speedup vs baseline: 1.2728x; 1.2728x over previous
# Linear-attention layer (phi = elu+1) on 8 Trainium2 NeuronCores.
#
# Reference computation (per batch b):
#   q = x @ Wq + bq ; k = x @ Wk + bk ; v = x @ Wv + bv      [S, DM] each
#   kv[h] = phi(k_h)^T @ v_h          (sum over ALL of S)    [HD, HD]
#   attn_h = phi(q_h) @ kv[h]                                [S, HD]
#   out = attn @ Wo + bo                                     [S, DM]
#
# Sharding: sequence-parallel. Core c owns S/8 = 512 positions of every
# batch (2048 rows total). kv is a sum over sequence -> each core computes
# a partial kv over its rows, a 0.5 MiB AllReduce combines them, and every
# core finishes its own rows through attn + out_proj. Output rows are
# disjoint across cores, so no other communication is needed.
#
# phi(t) = elu(t) + 1 = exp(min(t, 0)) + relu(t)
#
# Numerics: x/W cast to bf16 on host, matmuls accumulate in fp32 PSUM,
# kv state reduced in bf16 (validated ~3.5e-3 scaled absmax vs fp32 ref).

import numpy as np
import ml_dtypes

B, S, DM, H = 4, 4096, 1024, 16
HD = DM // H          # 64
N_CORES = 8
P = 128
SC = S // N_CORES     # 512 sequence positions per core
R = B * SC            # 2048 rows per core
KC = DM // P          # 8 contraction chunks
NB = SC               # rows per batch on a core (512)
SCB = NB // P         # s-chunks per batch (4)
SCH = R // P          # s-chunks total (16)
NT_R = R // 512       # 512-wide row tiles (4)
ND = DM // 512        # 512-wide feature tiles (2)

_cache = {}


def _build(has_bias):
    import concourse.mybir as mybir
    import concourse.tile as tile
    from concourse import bacc

    fp32 = mybir.dt.float32
    bf16 = mybir.dt.bfloat16
    AF = mybir.ActivationFunctionType
    ALU = mybir.AluOpType

    nc = bacc.Bacc("TRN2", target_bir_lowering=False, debug=False,
                   num_devices=N_CORES)

    x_d = nc.dram_tensor("x", [DM, R], bf16, kind="ExternalInput").ap()
    wq_d = nc.dram_tensor("wq", [DM, DM], bf16, kind="ExternalInput").ap()
    wk_d = nc.dram_tensor("wk", [DM, DM], bf16, kind="ExternalInput").ap()
    wv_d = nc.dram_tensor("wv", [DM, DM], bf16, kind="ExternalInput").ap()
    wo_d = nc.dram_tensor("wo", [DM, DM], bf16, kind="ExternalInput").ap()
    bqc_d = nc.dram_tensor("bqc", [P, KC], fp32, kind="ExternalInput").ap()
    if has_bias:
        bk_d = nc.dram_tensor("bk2", [1, DM], bf16, kind="ExternalInput").ap()
        bv_d = nc.dram_tensor("bv2", [1, DM], bf16, kind="ExternalInput").ap()
        bo_d = nc.dram_tensor("bo2", [1, DM], bf16, kind="ExternalInput").ap()
    out_d = nc.dram_tensor("out", [R, DM], fp32, kind="ExternalOutput").ap()
    if _cache.get("debug"):
        dkv_d = nc.dram_tensor("dkv", [P, B * (H // 2) * HD], fp32,
                               kind="ExternalOutput").ap()
        dkvr_d = nc.dram_tensor("dkvr", [P, B * (H // 2) * HD], fp32,
                                kind="ExternalOutput").ap()
        dphiq_d = nc.dram_tensor("dphiq", [P, KC * R], fp32,
                                 kind="ExternalOutput").ap()
        dattnt_d = nc.dram_tensor("dattnt", [P, KC * R], fp32,
                                  kind="ExternalOutput").ap()
        dxt_d = nc.dram_tensor("dxt", [P, KC * R], fp32,
                               kind="ExternalOutput").ap()

    with tile.TileContext(nc) as tc:
        with (
            tc.tile_pool(name="big", bufs=1) as big,
            tc.tile_pool(name="stream", bufs=3) as stream,
            tc.tile_pool(name="tmp", bufs=2) as tmpp,
            tc.tile_pool(name="outp", bufs=3) as outp,
            tc.tile_pool(name="psum", bufs=6, space="PSUM") as psum,
            tc.tile_pool(name="kvps", bufs=1, space="PSUM") as kvps,
            tc.tile_pool(name="dram", bufs=1, space="DRAM") as dram,
        ):
            # ---------------- persistent tiles ----------------
            # wk/wv live only through phase 1 (own pool, closed after);
            # attnt is allocated afterwards and reuses their space.
            wkv_pool = tc.tile_pool(name="wkv", bufs=1)
            wkv = wkv_pool.__enter__()
            xt = big.tile([P, KC, R], bf16, tag="xt")        # x^T
            wq = big.tile([P, KC, DM], bf16, tag="wq")
            wk = wkv.tile([P, KC, DM], bf16, tag="wk")
            wv = wkv.tile([P, KC, DM], bf16, tag="wv")
            wo = big.tile([P, KC, DM], bf16, tag="wo")
            phiq = big.tile([P, KC, R], bf16, tag="phiq")    # phi(q)^T
            # kv state: head-pair stacked on partitions (even head rows 0:64,
            # odd head rows 64:128); column slot (b*8 + pair)*64
            kv_sb = big.tile([P, B * (H // 2) * HD], bf16, tag="kv")
            kv_rd = big.tile([P, B * (H // 2) * HD], bf16, tag="kvr")
            bqc = big.tile([P, KC], fp32, tag="bqc")
            if has_bias:
                bk2 = big.tile([1, DM], bf16, tag="bk2")
                bv2 = big.tile([1, DM], bf16, tag="bv2")
                bo2 = big.tile([1, DM], bf16, tag="bo2")
            ones = big.tile([1, P], bf16, tag="ones")
            zrow = big.tile([1, 512], bf16, tag="zrow")

            kv_in = dram.tile([P, B * (H // 2) * HD], bf16)
            kv_out = dram.tile([P, B * (H // 2) * HD], bf16)

            def s512(n):
                return slice(n * 512, (n + 1) * 512)

            # ---------------- loads ----------------
            # x arrives pre-transposed from the host: plain contiguous
            # loads, row-group split so phase 1 unblocks early (sync
            # queue); weights on the scalar HWDGE queue in phase order.
            xt_dr = x_d.rearrange("(c p) r -> p c r", p=P)
            for rg in range(NT_R):
                nc.sync.dma_start(xt[:, :, s512(rg)], xt_dr[:, :, s512(rg)])
            for w_sb, w_dr in ((wk, wk_d), (wv, wv_d), (wq, wq_d), (wo, wo_d)):
                for c in range(KC):
                    nc.scalar.dma_start(
                        w_sb[:, c, :],
                        w_dr[c * P:(c + 1) * P, :])
            nc.gpsimd.dma_start(bqc[:], bqc_d)
            if has_bias:
                nc.gpsimd.dma_start(bk2[:], bk_d)
                nc.gpsimd.dma_start(bv2[:], bv_d)
                nc.gpsimd.dma_start(bo2[:], bo_d)
            nc.gpsimd.memset(ones[:], 1.0)
            nc.gpsimd.memset(zrow[:], 0.0)

            # ---------- phase 1: k/v projections + phi(k) + partial kv ----------
            for b in range(B):
                kvp = [kvps.tile([P, 512], fp32, tag=f"kvp{j}", name=f"kvp{j}")
                       for j in (0, 1)]
                for j in (0, 1):
                    # start=True zeroes the whole PSUM bank; do it exactly once
                    # per bank (full-width) so the per-head slot matmuls below
                    # can all accumulate with start=False.
                    nc.tensor.matmul(kvp[j][:], lhsT=ones[:], rhs=zrow[:],
                                     start=True, stop=False)
                for sc in range(SCB):
                    g = b * SCB + sc
                    kch = stream.tile([P, DM], bf16, tag="kch")
                    vch = stream.tile([P, DM], bf16, tag="vch")
                    for n in range(ND):
                        kps = psum.tile([P, 512], fp32, tag="pp")
                        vps = psum.tile([P, 512], fp32, tag="pp")
                        for kc in range(KC):
                            nc.tensor.matmul(
                                kps[:], lhsT=xt[:, kc, g * P:(g + 1) * P],
                                rhs=wk[:, kc, s512(n)],
                                start=(kc == 0),
                                stop=(not has_bias and kc == KC - 1))
                        if has_bias:
                            nc.tensor.matmul(kps[:], lhsT=ones[:],
                                             rhs=bk2[:, s512(n)],
                                             start=False, stop=True)
                        for kc in range(KC):
                            nc.tensor.matmul(
                                vps[:], lhsT=xt[:, kc, g * P:(g + 1) * P],
                                rhs=wv[:, kc, s512(n)],
                                start=(kc == 0),
                                stop=(not has_bias and kc == KC - 1))
                        if has_bias:
                            nc.tensor.matmul(vps[:], lhsT=ones[:],
                                             rhs=bv2[:, s512(n)],
                                             start=False, stop=True)
                        # phi(k) = exp(min(k,0)) + relu(k)
                        ut = tmpp.tile([P, 512], fp32, tag="u")
                        nc.vector.tensor_scalar_min(out=ut[:], in0=kps[:],
                                                    scalar1=0.0)
                        et = tmpp.tile([P, 512], bf16, tag="e")
                        nc.scalar.activation(out=et[:], in_=ut[:], func=AF.Exp)
                        rt = tmpp.tile([P, 512], bf16, tag="r")
                        nc.scalar.activation(out=rt[:], in_=kps[:], func=AF.Relu)
                        nc.vector.tensor_add(out=kch[:, s512(n)], in0=et[:],
                                             in1=rt[:])
                        nc.scalar.activation(out=vch[:, s512(n)], in_=vps[:],
                                             func=AF.Copy)
                    for h in range(H):
                        j, col = h // 8, (h % 8) * HD
                        # stationary = full head-pair block of phi(k); the
                        # off-pair half of the output is unused cross-terms
                        nc.tensor.matmul(
                            kvp[j][:, col:col + HD],
                            lhsT=kch[:, (h // 2) * P:(h // 2 + 1) * P],
                            rhs=vch[:, h * HD:(h + 1) * HD],
                            start=False,
                            stop=(sc == SCB - 1 and h % 8 == 7),
                            skip_group_check=True)
                for h in range(H):
                    j, col = h // 8, (h % 8) * HD
                    rows = slice((h % 2) * HD, (h % 2 + 1) * HD)
                    slot = (b * (H // 2) + h // 2) * HD
                    nc.scalar.activation(
                        out=kv_sb[rows, slot:slot + HD],
                        in_=kvp[j][rows, col:col + HD], func=AF.Copy)

            # wk/wv dead from here; free their SBUF for attnt
            wkv_pool.__exit__(None, None, None)
            attnt = big.tile([P, KC, R], bf16, tag="attnt")  # attn^T

            # ---------- phase 2: kv AllReduce (runs on TOPSP, overlaps phase 3) ----------
            nc.gpsimd.dma_start(kv_in[:], kv_sb[:])
            nc.gpsimd.collective_compute(
                "AllReduce",
                mybir.AluOpType.add,
                replica_groups=[list(range(N_CORES))],
                ins=[kv_in.opt()],
                outs=[kv_out.opt()],
            )
            nc.gpsimd.dma_start(kv_rd[:], kv_out[:])

            # ---------- phase 3: q^T projection + phi ----------
            for m in range(KC):
                for nt in range(NT_R):
                    qps = psum.tile([P, 512], fp32, tag="pp")
                    for kc in range(KC):
                        nc.tensor.matmul(
                            qps[:], lhsT=wq[:, kc, m * P:(m + 1) * P],
                            rhs=xt[:, kc, s512(nt)],
                            start=(kc == 0), stop=(kc == KC - 1))
                    ut = tmpp.tile([P, 512], fp32, tag="u")
                    nc.vector.tensor_scalar(out=ut[:], in0=qps[:],
                                            scalar1=bqc[:, m:m + 1],
                                            scalar2=0.0,
                                            op0=ALU.add, op1=ALU.min)
                    et = tmpp.tile([P, 512], bf16, tag="e")
                    nc.scalar.activation(out=et[:], in_=ut[:], func=AF.Exp)
                    rt = tmpp.tile([P, 512], bf16, tag="r")
                    nc.scalar.activation(out=rt[:], in_=qps[:], func=AF.Relu,
                                         bias=bqc[:, m:m + 1], scale=1.0)
                    nc.vector.tensor_add(out=phiq[:, m, s512(nt)], in0=et[:],
                                         in1=rt[:])

            # ---------- phase 4: attn^T = kv^T @ phi(q)^T per (b, head pair) ----------
            for b in range(B):
                for pr in range(H // 2):
                    ap = psum.tile([P, NB], fp32, tag="pp")
                    slot = (b * (H // 2) + pr) * HD
                    for half in range(2):
                        rows = slice(half * HD, (half + 1) * HD)
                        nc.tensor.matmul(
                            ap[rows, :],
                            lhsT=kv_rd[rows, slot:slot + HD],
                            rhs=phiq[rows, pr, b * NB:(b + 1) * NB],
                            start=True, stop=True)
                    nc.scalar.activation(
                        out=attnt[:, pr, b * NB:(b + 1) * NB],
                        in_=ap[:], func=AF.Copy)

            # ---------- phase 5: out = attn @ Wo + bo ----------
            for g in range(SCH):
                for n in range(ND):
                    ops = psum.tile([P, 512], fp32, tag="pp")
                    for kc in range(KC):
                        nc.tensor.matmul(
                            ops[:], lhsT=attnt[:, kc, g * P:(g + 1) * P],
                            rhs=wo[:, kc, s512(n)],
                            start=(kc == 0),
                            stop=(not has_bias and kc == KC - 1))
                    if has_bias:
                        nc.tensor.matmul(ops[:], lhsT=ones[:],
                                         rhs=bo2[:, s512(n)],
                                         start=False, stop=True)
                    osb = outp.tile([P, 512], fp32, tag="osb")
                    nc.scalar.activation(out=osb[:], in_=ops[:], func=AF.Copy)
                    nc.sync.dma_start(out_d[g * P:(g + 1) * P, s512(n)], osb[:])

            if _cache.get("debug"):
                for src_t, dst in ((kv_sb, dkv_d), (kv_rd, dkvr_d)):
                    for n in range(4):
                        f32t = outp.tile([P, 512], fp32, tag="osb")
                        nc.vector.tensor_copy(out=f32t[:], in_=src_t[:, s512(n)])
                        nc.sync.dma_start(dst[:, s512(n)], f32t[:])
                for src_t, dst in ((phiq, dphiq_d), (attnt, dattnt_d), (xt, dxt_d)):
                    for m in range(KC):
                        for n in range(NT_R):
                            f32t = outp.tile([P, 512], fp32, tag="osb")
                            nc.vector.tensor_copy(out=f32t[:],
                                                  in_=src_t[:, m, s512(n)])
                            nc.sync.dma_start(dst[:, m * R + n * 512:
                                                  m * R + (n + 1) * 512], f32t[:])

    nc.compile()
    return nc


def _get_nc(has_bias):
    key = ("nc", has_bias)
    if key not in _cache:
        _cache[key] = _build(has_bias)
    return _cache[key]


def _has_bias(inputs):
    return any(np.any(np.asarray(inputs[k], np.float32))
               for k in ("bk", "bv", "bo"))


def _make_in_maps(inputs, has_bias):
    bf16 = ml_dtypes.bfloat16
    x = np.asarray(inputs["x"], dtype=np.float32)
    ws = {k: np.ascontiguousarray(np.asarray(inputs[k], np.float32).astype(bf16))
          for k in ("Wq", "Wk", "Wv", "Wo")}
    bq = np.asarray(inputs["bq"], np.float32)
    bqc = np.ascontiguousarray(bq.reshape(KC, P).T.astype(np.float32))
    brow = {k: np.ascontiguousarray(
                np.asarray(inputs[k], np.float32).astype(bf16).reshape(1, DM))
            for k in ("bk", "bv", "bo")}
    xb = x.astype(bf16)
    in_maps = []
    for c in range(N_CORES):
        xs = np.ascontiguousarray(
            xb[:, c * SC:(c + 1) * SC, :].reshape(R, DM).T)
        m = {
            "x": xs,
            "wq": ws["Wq"], "wk": ws["Wk"], "wv": ws["Wv"], "wo": ws["Wo"],
            "bqc": bqc,
        }
        if has_bias:
            m.update({"bk2": brow["bk"], "bv2": brow["bv"],
                      "bo2": brow["bo"]})
        in_maps.append(m)
    return in_maps


def _run(inputs, **kw):
    from concourse import bass_utils
    hb = _has_bias(inputs)
    nc = _get_nc(hb)
    in_maps = _make_in_maps(inputs, hb)
    res = bass_utils.run_bass_kernel_spmd(
        nc, in_maps, core_ids=list(range(N_CORES)), **kw)
    out = np.empty((B, S, DM), np.float32)
    for c in range(N_CORES):
        out[:, c * SC:(c + 1) * SC, :] = res.results[c]["out"].reshape(B, SC, DM)
    return out, res


def kernel(**inputs) -> np.ndarray:
    out, _ = _run(inputs)
    return out


# revision 18
# speedup vs baseline: 1.2831x; 1.0081x over previous
# Linear-attention layer (phi = elu+1) on 8 Trainium2 NeuronCores.
#
# Reference computation (per batch b):
#   q = x @ Wq + bq ; k = x @ Wk + bk ; v = x @ Wv + bv      [S, DM] each
#   kv[h] = phi(k_h)^T @ v_h          (sum over ALL of S)    [HD, HD]
#   attn_h = phi(q_h) @ kv[h]                                [S, HD]
#   out = attn @ Wo + bo                                     [S, DM]
#
# Sharding: sequence-parallel. Core c owns S/8 = 512 positions of every
# batch (2048 rows total). kv is a sum over sequence -> each core computes
# a partial kv over its rows, a 0.5 MiB AllReduce combines them, and every
# core finishes its own rows through attn + out_proj. Output rows are
# disjoint across cores, so no other communication is needed.
#
# phi(t) = elu(t) + 1 = exp(min(t, 0)) + relu(t)
#
# Numerics: x/W cast to bf16 on host, matmuls accumulate in fp32 PSUM,
# kv state reduced in bf16 (validated ~3.5e-3 scaled absmax vs fp32 ref).

import numpy as np
import ml_dtypes

B, S, DM, H = 4, 4096, 1024, 16
HD = DM // H          # 64
N_CORES = 8
P = 128
SC = S // N_CORES     # 512 sequence positions per core
R = B * SC            # 2048 rows per core
KC = DM // P          # 8 contraction chunks
NB = SC               # rows per batch on a core (512)
SCB = NB // P         # s-chunks per batch (4)
SCH = R // P          # s-chunks total (16)
NT_R = R // 512       # 512-wide row tiles (4)
ND = DM // 512        # 512-wide feature tiles (2)

_cache = {}


def _build(has_bias):
    import concourse.mybir as mybir
    import concourse.tile as tile
    from concourse import bacc

    fp32 = mybir.dt.float32
    bf16 = mybir.dt.bfloat16
    AF = mybir.ActivationFunctionType
    ALU = mybir.AluOpType

    nc = bacc.Bacc("TRN2", target_bir_lowering=False, debug=False,
                   num_devices=N_CORES)

    x_d = nc.dram_tensor("x", [DM, R], bf16, kind="ExternalInput").ap()
    wq_d = nc.dram_tensor("wq", [DM, DM], bf16, kind="ExternalInput").ap()
    wk_d = nc.dram_tensor("wk", [DM, DM], bf16, kind="ExternalInput").ap()
    wv_d = nc.dram_tensor("wv", [DM, DM], bf16, kind="ExternalInput").ap()
    wo_d = nc.dram_tensor("wo", [DM, DM], bf16, kind="ExternalInput").ap()
    bqc_d = nc.dram_tensor("bqc", [P, KC], fp32, kind="ExternalInput").ap()
    if has_bias:
        bk_d = nc.dram_tensor("bk2", [1, DM], bf16, kind="ExternalInput").ap()
        bv_d = nc.dram_tensor("bv2", [1, DM], bf16, kind="ExternalInput").ap()
        bo_d = nc.dram_tensor("bo2", [1, DM], bf16, kind="ExternalInput").ap()
    out_d = nc.dram_tensor("out", [R, DM], fp32, kind="ExternalOutput").ap()
    if _cache.get("debug"):
        dkv_d = nc.dram_tensor("dkv", [P, B * (H // 2) * HD], fp32,
                               kind="ExternalOutput").ap()
        dkvr_d = nc.dram_tensor("dkvr", [P, B * (H // 2) * HD], fp32,
                                kind="ExternalOutput").ap()
        dphiq_d = nc.dram_tensor("dphiq", [P, KC * R], fp32,
                                 kind="ExternalOutput").ap()
        dattnt_d = nc.dram_tensor("dattnt", [P, KC * R], fp32,
                                  kind="ExternalOutput").ap()
        dxt_d = nc.dram_tensor("dxt", [P, KC * R], fp32,
                               kind="ExternalOutput").ap()

    with tile.TileContext(nc) as tc:
        with (
            tc.tile_pool(name="big", bufs=1) as big,
            tc.tile_pool(name="stream", bufs=3) as stream,
            tc.tile_pool(name="tmp", bufs=2) as tmpp,
            tc.tile_pool(name="outp", bufs=3) as outp,
            tc.tile_pool(name="psum", bufs=4, space="PSUM") as psum,
            tc.tile_pool(name="kvps", bufs=2, space="PSUM") as kvps,
            tc.tile_pool(name="dram", bufs=1, space="DRAM") as dram,
        ):
            # ---------------- persistent tiles ----------------
            # wk/wv live only through phase 1 (own pool, closed after);
            # attnt is allocated afterwards and reuses their space.
            wkv_pool = tc.tile_pool(name="wkv", bufs=1)
            wkv = wkv_pool.__enter__()
            xt = big.tile([P, KC, R], bf16, tag="xt")        # x^T
            wq = big.tile([P, KC, DM], bf16, tag="wq")
            wk = wkv.tile([P, KC, DM], bf16, tag="wk")
            wv = wkv.tile([P, KC, DM], bf16, tag="wv")
            wo = big.tile([P, KC, DM], bf16, tag="wo")
            phiq = big.tile([P, KC, R], bf16, tag="phiq")    # phi(q)^T
            # kv state: head-pair stacked on partitions (even head rows 0:64,
            # odd head rows 64:128); column slot (b*8 + pair)*64
            kv_sb = big.tile([P, B * (H // 2) * HD], bf16, tag="kv")
            kv_rd = big.tile([P, B * (H // 2) * HD], bf16, tag="kvr")
            bqc = big.tile([P, KC], fp32, tag="bqc")
            if has_bias:
                bk2 = big.tile([1, DM], bf16, tag="bk2")
                bv2 = big.tile([1, DM], bf16, tag="bv2")
                bo2 = big.tile([1, DM], bf16, tag="bo2")
            ones = big.tile([1, P], bf16, tag="ones")
            zrow = big.tile([1, 512], bf16, tag="zrow")

            kv_in = dram.tile([P, B * (H // 2) * HD], bf16)
            kv_out = dram.tile([P, B * (H // 2) * HD], bf16)

            def s512(n):
                return slice(n * 512, (n + 1) * 512)

            # ---------------- loads ----------------
            # x arrives pre-transposed from the host: plain contiguous
            # loads, row-group split so phase 1 unblocks early (sync
            # queue); weights on the scalar HWDGE queue in phase order.
            xt_dr = x_d.rearrange("(c p) r -> p c r", p=P)
            for rg in range(NT_R):
                nc.sync.dma_start(xt[:, :, s512(rg)], xt_dr[:, :, s512(rg)])
            for w_sb, w_dr in ((wk, wk_d), (wv, wv_d), (wq, wq_d), (wo, wo_d)):
                for c in range(KC):
                    nc.scalar.dma_start(
                        w_sb[:, c, :],
                        w_dr[c * P:(c + 1) * P, :])
            nc.gpsimd.dma_start(bqc[:], bqc_d)
            if has_bias:
                nc.gpsimd.dma_start(bk2[:], bk_d)
                nc.gpsimd.dma_start(bv2[:], bv_d)
                nc.gpsimd.dma_start(bo2[:], bo_d)
            nc.gpsimd.memset(ones[:], 1.0)
            nc.gpsimd.memset(zrow[:], 0.0)

            # ---------- phase 1: k/v projections + phi(k) + partial kv ----------
            for b in range(B):
                kvp = [kvps.tile([P, 512], fp32, tag=f"kvp{j}", name=f"kvp{j}")
                       for j in (0, 1)]
                for j in (0, 1):
                    # start=True zeroes the whole PSUM bank; do it exactly once
                    # per bank (full-width) so the per-head slot matmuls below
                    # can all accumulate with start=False.
                    nc.tensor.matmul(kvp[j][:], lhsT=ones[:], rhs=zrow[:],
                                     start=True, stop=False)
                for sc in range(SCB):
                    g = b * SCB + sc
                    kch = stream.tile([P, DM], bf16, tag="kch")
                    vch = stream.tile([P, DM], bf16, tag="vch")
                    for n in range(ND):
                        kps = psum.tile([P, 512], fp32, tag="pp")
                        vps = psum.tile([P, 512], fp32, tag="pp")
                        for kc in range(KC):
                            nc.tensor.matmul(
                                kps[:], lhsT=xt[:, kc, g * P:(g + 1) * P],
                                rhs=wk[:, kc, s512(n)],
                                start=(kc == 0),
                                stop=(not has_bias and kc == KC - 1))
                        if has_bias:
                            nc.tensor.matmul(kps[:], lhsT=ones[:],
                                             rhs=bk2[:, s512(n)],
                                             start=False, stop=True)
                        for kc in range(KC):
                            nc.tensor.matmul(
                                vps[:], lhsT=xt[:, kc, g * P:(g + 1) * P],
                                rhs=wv[:, kc, s512(n)],
                                start=(kc == 0),
                                stop=(not has_bias and kc == KC - 1))
                        if has_bias:
                            nc.tensor.matmul(vps[:], lhsT=ones[:],
                                             rhs=bv2[:, s512(n)],
                                             start=False, stop=True)
                        # phi(k) = exp(min(k,0)) + relu(k)
                        ut = tmpp.tile([P, 512], fp32, tag="u")
                        nc.vector.tensor_scalar_min(out=ut[:], in0=kps[:],
                                                    scalar1=0.0)
                        et = tmpp.tile([P, 512], bf16, tag="e")
                        nc.scalar.activation(out=et[:], in_=ut[:], func=AF.Exp)
                        rt = tmpp.tile([P, 512], bf16, tag="r")
                        nc.scalar.activation(out=rt[:], in_=kps[:], func=AF.Relu)
                        nc.vector.tensor_add(out=kch[:, s512(n)], in0=et[:],
                                             in1=rt[:])
                        nc.vector.tensor_copy(out=vch[:, s512(n)], in_=vps[:])
                    for h in range(H):
                        j, col = h // 8, (h % 8) * HD
                        # stationary = full head-pair block of phi(k); the
                        # off-pair half of the output is unused cross-terms
                        nc.tensor.matmul(
                            kvp[j][:, col:col + HD],
                            lhsT=kch[:, (h // 2) * P:(h // 2 + 1) * P],
                            rhs=vch[:, h * HD:(h + 1) * HD],
                            start=False,
                            stop=(sc == SCB - 1 and h % 8 == 7),
                            skip_group_check=True)
                for h in range(H):
                    j, col = h // 8, (h % 8) * HD
                    rows = slice((h % 2) * HD, (h % 2 + 1) * HD)
                    slot = (b * (H // 2) + h // 2) * HD
                    nc.vector.tensor_copy(
                        out=kv_sb[rows, slot:slot + HD],
                        in_=kvp[j][rows, col:col + HD])

            # wk/wv dead from here; free their SBUF for attnt
            wkv_pool.__exit__(None, None, None)
            attnt = big.tile([P, KC, R], bf16, tag="attnt")  # attn^T

            # ---------- phase 2: kv AllReduce (runs on TOPSP, overlaps phase 3) ----------
            nc.gpsimd.dma_start(kv_in[:], kv_sb[:])
            nc.gpsimd.collective_compute(
                "AllReduce",
                mybir.AluOpType.add,
                replica_groups=[list(range(N_CORES))],
                ins=[kv_in.opt()],
                outs=[kv_out.opt()],
            )
            nc.gpsimd.dma_start(kv_rd[:], kv_out[:])

            # ---------- phase 3: q^T projection + phi ----------
            for m in range(KC):
                for nt in range(NT_R):
                    qps = psum.tile([P, 512], fp32, tag="pp")
                    for kc in range(KC):
                        nc.tensor.matmul(
                            qps[:], lhsT=wq[:, kc, m * P:(m + 1) * P],
                            rhs=xt[:, kc, s512(nt)],
                            start=(kc == 0), stop=(kc == KC - 1))
                    ut = tmpp.tile([P, 512], fp32, tag="u")
                    nc.vector.tensor_scalar(out=ut[:], in0=qps[:],
                                            scalar1=bqc[:, m:m + 1],
                                            scalar2=0.0,
                                            op0=ALU.add, op1=ALU.min)
                    et = tmpp.tile([P, 512], bf16, tag="e")
                    nc.scalar.activation(out=et[:], in_=ut[:], func=AF.Exp)
                    rt = tmpp.tile([P, 512], bf16, tag="r")
                    nc.scalar.activation(out=rt[:], in_=qps[:], func=AF.Relu,
                                         bias=bqc[:, m:m + 1], scale=1.0)
                    nc.vector.tensor_add(out=phiq[:, m, s512(nt)], in0=et[:],
                                         in1=rt[:])

            # ---------- phase 4: attn^T = kv^T @ phi(q)^T per (b, head pair) ----------
            for b in range(B):
                for pr in range(H // 2):
                    ap = psum.tile([P, NB], fp32, tag="pp")
                    slot = (b * (H // 2) + pr) * HD
                    for half in range(2):
                        rows = slice(half * HD, (half + 1) * HD)
                        nc.tensor.matmul(
                            ap[rows, :],
                            lhsT=kv_rd[rows, slot:slot + HD],
                            rhs=phiq[rows, pr, b * NB:(b + 1) * NB],
                            start=True, stop=True)
                    nc.scalar.activation(
                        out=attnt[:, pr, b * NB:(b + 1) * NB],
                        in_=ap[:], func=AF.Copy)

            # ---------- phase 5: out = attn @ Wo + bo ----------
            for g in range(SCH):
                for n in range(ND):
                    ops = psum.tile([P, 512], fp32, tag="pp")
                    for kc in range(KC):
                        nc.tensor.matmul(
                            ops[:], lhsT=attnt[:, kc, g * P:(g + 1) * P],
                            rhs=wo[:, kc, s512(n)],
                            start=(kc == 0),
                            stop=(not has_bias and kc == KC - 1))
                    if has_bias:
                        nc.tensor.matmul(ops[:], lhsT=ones[:],
                                         rhs=bo2[:, s512(n)],
                                         start=False, stop=True)
                    osb = outp.tile([P, 512], fp32, tag="osb")
                    nc.scalar.activation(out=osb[:], in_=ops[:], func=AF.Copy)
                    nc.sync.dma_start(out_d[g * P:(g + 1) * P, s512(n)], osb[:])

            if _cache.get("debug"):
                for src_t, dst in ((kv_sb, dkv_d), (kv_rd, dkvr_d)):
                    for n in range(4):
                        f32t = outp.tile([P, 512], fp32, tag="osb")
                        nc.vector.tensor_copy(out=f32t[:], in_=src_t[:, s512(n)])
                        nc.sync.dma_start(dst[:, s512(n)], f32t[:])
                for src_t, dst in ((phiq, dphiq_d), (attnt, dattnt_d), (xt, dxt_d)):
                    for m in range(KC):
                        for n in range(NT_R):
                            f32t = outp.tile([P, 512], fp32, tag="osb")
                            nc.vector.tensor_copy(out=f32t[:],
                                                  in_=src_t[:, m, s512(n)])
                            nc.sync.dma_start(dst[:, m * R + n * 512:
                                                  m * R + (n + 1) * 512], f32t[:])

    nc.compile()
    return nc


def _get_nc(has_bias):
    key = ("nc", has_bias)
    if key not in _cache:
        _cache[key] = _build(has_bias)
    return _cache[key]


def _has_bias(inputs):
    return any(np.any(np.asarray(inputs[k], np.float32))
               for k in ("bk", "bv", "bo"))


def _make_in_maps(inputs, has_bias):
    bf16 = ml_dtypes.bfloat16
    x = np.asarray(inputs["x"], dtype=np.float32)
    ws = {k: np.ascontiguousarray(np.asarray(inputs[k], np.float32).astype(bf16))
          for k in ("Wq", "Wk", "Wv", "Wo")}
    bq = np.asarray(inputs["bq"], np.float32)
    bqc = np.ascontiguousarray(bq.reshape(KC, P).T.astype(np.float32))
    brow = {k: np.ascontiguousarray(
                np.asarray(inputs[k], np.float32).astype(bf16).reshape(1, DM))
            for k in ("bk", "bv", "bo")}
    xb = x.astype(bf16)
    in_maps = []
    for c in range(N_CORES):
        xs = np.ascontiguousarray(
            xb[:, c * SC:(c + 1) * SC, :].reshape(R, DM).T)
        m = {
            "x": xs,
            "wq": ws["Wq"], "wk": ws["Wk"], "wv": ws["Wv"], "wo": ws["Wo"],
            "bqc": bqc,
        }
        if has_bias:
            m.update({"bk2": brow["bk"], "bv2": brow["bv"],
                      "bo2": brow["bo"]})
        in_maps.append(m)
    return in_maps


def _run(inputs, **kw):
    from concourse import bass_utils
    hb = _has_bias(inputs)
    nc = _get_nc(hb)
    in_maps = _make_in_maps(inputs, hb)
    res = bass_utils.run_bass_kernel_spmd(
        nc, in_maps, core_ids=list(range(N_CORES)), **kw)
    out = np.empty((B, S, DM), np.float32)
    for c in range(N_CORES):
        out[:, c * SC:(c + 1) * SC, :] = res.results[c]["out"].reshape(B, SC, DM)
    return out, res


def kernel(**inputs) -> np.ndarray:
    out, _ = _run(inputs)
    return out


# revision 19
# speedup vs baseline: 1.3036x; 1.0159x over previous
# Linear-attention layer (phi = elu+1) on 8 Trainium2 NeuronCores.
#
# Reference computation (per batch b):
#   q = x @ Wq + bq ; k = x @ Wk + bk ; v = x @ Wv + bv      [S, DM] each
#   kv[h] = phi(k_h)^T @ v_h          (sum over ALL of S)    [HD, HD]
#   attn_h = phi(q_h) @ kv[h]                                [S, HD]
#   out = attn @ Wo + bo                                     [S, DM]
#
# Sharding: sequence-parallel. Core c owns S/8 = 512 positions of every
# batch (2048 rows total). kv is a sum over sequence -> each core computes
# a partial kv over its rows, a 0.5 MiB AllReduce combines them, and every
# core finishes its own rows through attn + out_proj. Output rows are
# disjoint across cores, so no other communication is needed.
#
# phi(t) = elu(t) + 1 = exp(min(t, 0)) + relu(t)
#
# Numerics: x/W cast to bf16 on host, matmuls accumulate in fp32 PSUM,
# kv state reduced in bf16 (validated ~3.5e-3 scaled absmax vs fp32 ref).

import numpy as np
import ml_dtypes

B, S, DM, H = 4, 4096, 1024, 16
HD = DM // H          # 64
N_CORES = 8
P = 128
SC = S // N_CORES     # 512 sequence positions per core
R = B * SC            # 2048 rows per core
KC = DM // P          # 8 contraction chunks
NB = SC               # rows per batch on a core (512)
SCB = NB // P         # s-chunks per batch (4)
SCH = R // P          # s-chunks total (16)
NT_R = R // 512       # 512-wide row tiles (4)
ND = DM // 512        # 512-wide feature tiles (2)

_cache = {}


def _build(has_bias):
    import concourse.mybir as mybir
    import concourse.tile as tile
    from concourse import bacc

    fp32 = mybir.dt.float32
    bf16 = mybir.dt.bfloat16
    AF = mybir.ActivationFunctionType
    ALU = mybir.AluOpType

    nc = bacc.Bacc("TRN2", target_bir_lowering=False, debug=False,
                   num_devices=N_CORES)

    x_d = nc.dram_tensor("x", [DM, R], bf16, kind="ExternalInput").ap()
    wq_d = nc.dram_tensor("wq", [DM, DM], bf16, kind="ExternalInput").ap()
    wk_d = nc.dram_tensor("wk", [DM, DM], bf16, kind="ExternalInput").ap()
    wv_d = nc.dram_tensor("wv", [DM, DM], bf16, kind="ExternalInput").ap()
    wo_d = nc.dram_tensor("wo", [DM, DM], bf16, kind="ExternalInput").ap()
    bqc_d = nc.dram_tensor("bqc", [P, KC], fp32, kind="ExternalInput").ap()
    if has_bias:
        bk_d = nc.dram_tensor("bk2", [1, DM], bf16, kind="ExternalInput").ap()
        bv_d = nc.dram_tensor("bv2", [1, DM], bf16, kind="ExternalInput").ap()
        bo_d = nc.dram_tensor("bo2", [1, DM], bf16, kind="ExternalInput").ap()
    out_d = nc.dram_tensor("out", [R, DM], fp32, kind="ExternalOutput").ap()
    if _cache.get("debug"):
        dkv_d = nc.dram_tensor("dkv", [P, B * (H // 2) * HD], fp32,
                               kind="ExternalOutput").ap()
        dkvr_d = nc.dram_tensor("dkvr", [P, B * (H // 2) * HD], fp32,
                                kind="ExternalOutput").ap()
        dphiq_d = nc.dram_tensor("dphiq", [P, KC * R], fp32,
                                 kind="ExternalOutput").ap()
        dattnt_d = nc.dram_tensor("dattnt", [P, KC * R], fp32,
                                  kind="ExternalOutput").ap()
        dxt_d = nc.dram_tensor("dxt", [P, KC * R], fp32,
                               kind="ExternalOutput").ap()

    with tile.TileContext(nc) as tc:
        with (
            tc.tile_pool(name="big", bufs=1) as big,
            tc.tile_pool(name="stream", bufs=3) as stream,
            tc.tile_pool(name="tmp", bufs=2) as tmpp,
            tc.tile_pool(name="outp", bufs=3) as outp,
            tc.tile_pool(name="psum", bufs=5, space="PSUM") as psum,
            tc.tile_pool(name="kvps", bufs=2, space="PSUM") as kvps,
            tc.tile_pool(name="dram", bufs=1, space="DRAM") as dram,
        ):
            # ---------------- persistent tiles ----------------
            # wk/wv live only through phase 1 (own pool, closed after);
            # attnt is allocated afterwards and reuses their space.
            wkv_pool = tc.tile_pool(name="wkv", bufs=1)
            wkv = wkv_pool.__enter__()
            xt = big.tile([P, KC, R], bf16, tag="xt")        # x^T
            wq = big.tile([P, KC, DM], bf16, tag="wq")
            wk = wkv.tile([P, KC, DM], bf16, tag="wk")
            wv = wkv.tile([P, KC, DM], bf16, tag="wv")
            wo = big.tile([P, KC, DM], bf16, tag="wo")
            phiq = big.tile([P, KC, R], bf16, tag="phiq")    # phi(q)^T
            # kv state: head-pair stacked on partitions (even head rows 0:64,
            # odd head rows 64:128); column slot (b*8 + pair)*64
            kv_sb = big.tile([P, B * (H // 2) * HD], bf16, tag="kv")
            kv_rd = big.tile([P, B * (H // 2) * HD], bf16, tag="kvr")
            bqc = big.tile([P, KC], fp32, tag="bqc")
            if has_bias:
                bk2 = big.tile([1, DM], bf16, tag="bk2")
                bv2 = big.tile([1, DM], bf16, tag="bv2")
                bo2 = big.tile([1, DM], bf16, tag="bo2")
            ones = big.tile([1, P], bf16, tag="ones")
            zrow = big.tile([1, 512], bf16, tag="zrow")

            kv_in = dram.tile([P, B * (H // 2) * HD], bf16)
            kv_out = dram.tile([P, B * (H // 2) * HD], bf16)

            def s512(n):
                return slice(n * 512, (n + 1) * 512)

            # ---------------- loads ----------------
            # x arrives pre-transposed from the host: plain contiguous
            # loads, row-group split so phase 1 unblocks early (sync
            # queue); weights on the scalar HWDGE queue in phase order.
            xt_dr = x_d.rearrange("(c p) r -> p c r", p=P)
            for rg in range(NT_R):
                nc.sync.dma_start(xt[:, :, s512(rg)], xt_dr[:, :, s512(rg)])
            for w_sb, w_dr in ((wk, wk_d), (wv, wv_d), (wq, wq_d), (wo, wo_d)):
                for c in range(KC):
                    nc.scalar.dma_start(
                        w_sb[:, c, :],
                        w_dr[c * P:(c + 1) * P, :])
            nc.gpsimd.dma_start(bqc[:], bqc_d)
            if has_bias:
                nc.gpsimd.dma_start(bk2[:], bk_d)
                nc.gpsimd.dma_start(bv2[:], bv_d)
                nc.gpsimd.dma_start(bo2[:], bo_d)
            nc.gpsimd.memset(ones[:], 1.0)
            nc.gpsimd.memset(zrow[:], 0.0)

            # ---------- phase 1: k/v projections + phi(k) + partial kv ----------
            for b in range(B):
                kvp = [kvps.tile([P, 512], fp32, tag="kvp0", name="kvp0",
                                 bufs=2),
                       kvps.tile([P, 512], fp32, tag="kvp1", name="kvp1",
                                 bufs=1)]
                for j in (0, 1):
                    # start=True zeroes the whole PSUM bank; do it exactly once
                    # per bank (full-width) so the per-head slot matmuls below
                    # can all accumulate with start=False.
                    nc.tensor.matmul(kvp[j][:], lhsT=ones[:], rhs=zrow[:],
                                     start=True, stop=False)
                for sc in range(SCB):
                    g = b * SCB + sc
                    kch = stream.tile([P, DM], bf16, tag="kch")
                    vch = stream.tile([P, DM], bf16, tag="vch")
                    for n in range(ND):
                        kps = psum.tile([P, 512], fp32, tag="pp")
                        vps = psum.tile([P, 512], fp32, tag="pp")
                        for kc in range(KC):
                            nc.tensor.matmul(
                                kps[:], lhsT=xt[:, kc, g * P:(g + 1) * P],
                                rhs=wk[:, kc, s512(n)],
                                start=(kc == 0),
                                stop=(not has_bias and kc == KC - 1))
                        if has_bias:
                            nc.tensor.matmul(kps[:], lhsT=ones[:],
                                             rhs=bk2[:, s512(n)],
                                             start=False, stop=True)
                        for kc in range(KC):
                            nc.tensor.matmul(
                                vps[:], lhsT=xt[:, kc, g * P:(g + 1) * P],
                                rhs=wv[:, kc, s512(n)],
                                start=(kc == 0),
                                stop=(not has_bias and kc == KC - 1))
                        if has_bias:
                            nc.tensor.matmul(vps[:], lhsT=ones[:],
                                             rhs=bv2[:, s512(n)],
                                             start=False, stop=True)
                        # phi(k) = exp(min(k,0)) + relu(k)
                        ut = tmpp.tile([P, 512], fp32, tag="u")
                        nc.vector.tensor_scalar_min(out=ut[:], in0=kps[:],
                                                    scalar1=0.0)
                        rt = tmpp.tile([P, 512], bf16, tag="r")
                        nc.scalar.activation(out=rt[:], in_=kps[:], func=AF.Relu)
                        nc.vector.tensor_copy(out=vch[:, s512(n)], in_=vps[:])
                        et = tmpp.tile([P, 512], bf16, tag="e")
                        nc.scalar.activation(out=et[:], in_=ut[:], func=AF.Exp)
                        nc.vector.tensor_add(out=kch[:, s512(n)], in0=et[:],
                                             in1=rt[:])
                    for h in range(H):
                        j, col = h // 8, (h % 8) * HD
                        # stationary = full head-pair block of phi(k); the
                        # off-pair half of the output is unused cross-terms
                        nc.tensor.matmul(
                            kvp[j][:, col:col + HD],
                            lhsT=kch[:, (h // 2) * P:(h // 2 + 1) * P],
                            rhs=vch[:, h * HD:(h + 1) * HD],
                            start=False,
                            stop=(sc == SCB - 1 and h % 8 == 7),
                            skip_group_check=True)
                for h in range(H):
                    j, col = h // 8, (h % 8) * HD
                    rows = slice((h % 2) * HD, (h % 2 + 1) * HD)
                    slot = (b * (H // 2) + h // 2) * HD
                    nc.vector.tensor_copy(
                        out=kv_sb[rows, slot:slot + HD],
                        in_=kvp[j][rows, col:col + HD])

            # wk/wv dead from here; free their SBUF for attnt
            wkv_pool.__exit__(None, None, None)
            attnt = big.tile([P, KC, R], bf16, tag="attnt")  # attn^T

            # ---------- phase 2: kv AllReduce (runs on TOPSP, overlaps phase 3) ----------
            nc.gpsimd.dma_start(kv_in[:], kv_sb[:])
            nc.gpsimd.collective_compute(
                "AllReduce",
                mybir.AluOpType.add,
                replica_groups=[list(range(N_CORES))],
                ins=[kv_in.opt()],
                outs=[kv_out.opt()],
            )
            nc.gpsimd.dma_start(kv_rd[:], kv_out[:])

            # ---------- phase 3: q^T projection + phi ----------
            for m in range(KC):
                for nt in range(NT_R):
                    qps = psum.tile([P, 512], fp32, tag="pp")
                    for kc in range(KC):
                        nc.tensor.matmul(
                            qps[:], lhsT=wq[:, kc, m * P:(m + 1) * P],
                            rhs=xt[:, kc, s512(nt)],
                            start=(kc == 0), stop=(kc == KC - 1))
                    ut = tmpp.tile([P, 512], fp32, tag="u")
                    nc.vector.tensor_scalar(out=ut[:], in0=qps[:],
                                            scalar1=bqc[:, m:m + 1],
                                            scalar2=0.0,
                                            op0=ALU.add, op1=ALU.min)
                    et = tmpp.tile([P, 512], bf16, tag="e")
                    nc.scalar.activation(out=et[:], in_=ut[:], func=AF.Exp)
                    rt = tmpp.tile([P, 512], bf16, tag="r")
                    nc.scalar.activation(out=rt[:], in_=qps[:], func=AF.Relu,
                                         bias=bqc[:, m:m + 1], scale=1.0)
                    nc.vector.tensor_add(out=phiq[:, m, s512(nt)], in0=et[:],
                                         in1=rt[:])

            # ---------- phase 4: attn^T = kv^T @ phi(q)^T per (b, head pair) ----------
            for b in range(B):
                for pr in range(H // 2):
                    ap = psum.tile([P, NB], fp32, tag="pp")
                    slot = (b * (H // 2) + pr) * HD
                    for half in range(2):
                        rows = slice(half * HD, (half + 1) * HD)
                        nc.tensor.matmul(
                            ap[rows, :],
                            lhsT=kv_rd[rows, slot:slot + HD],
                            rhs=phiq[rows, pr, b * NB:(b + 1) * NB],
                            start=True, stop=True)
                    nc.scalar.activation(
                        out=attnt[:, pr, b * NB:(b + 1) * NB],
                        in_=ap[:], func=AF.Copy)

            # ---------- phase 5: out = attn @ Wo + bo ----------
            for g in range(SCH):
                for n in range(ND):
                    ops = psum.tile([P, 512], fp32, tag="pp")
                    for kc in range(KC):
                        nc.tensor.matmul(
                            ops[:], lhsT=attnt[:, kc, g * P:(g + 1) * P],
                            rhs=wo[:, kc, s512(n)],
                            start=(kc == 0),
                            stop=(not has_bias and kc == KC - 1))
                    if has_bias:
                        nc.tensor.matmul(ops[:], lhsT=ones[:],
                                         rhs=bo2[:, s512(n)],
                                         start=False, stop=True)
                    osb = outp.tile([P, 512], fp32, tag="osb")
                    nc.scalar.activation(out=osb[:], in_=ops[:], func=AF.Copy)
                    nc.sync.dma_start(out_d[g * P:(g + 1) * P, s512(n)], osb[:])

            if _cache.get("debug"):
                for src_t, dst in ((kv_sb, dkv_d), (kv_rd, dkvr_d)):
                    for n in range(4):
                        f32t = outp.tile([P, 512], fp32, tag="osb")
                        nc.vector.tensor_copy(out=f32t[:], in_=src_t[:, s512(n)])
                        nc.sync.dma_start(dst[:, s512(n)], f32t[:])
                for src_t, dst in ((phiq, dphiq_d), (attnt, dattnt_d), (xt, dxt_d)):
                    for m in range(KC):
                        for n in range(NT_R):
                            f32t = outp.tile([P, 512], fp32, tag="osb")
                            nc.vector.tensor_copy(out=f32t[:],
                                                  in_=src_t[:, m, s512(n)])
                            nc.sync.dma_start(dst[:, m * R + n * 512:
                                                  m * R + (n + 1) * 512], f32t[:])

    nc.compile()
    return nc


def _get_nc(has_bias):
    key = ("nc", has_bias)
    if key not in _cache:
        _cache[key] = _build(has_bias)
    return _cache[key]


def _has_bias(inputs):
    return any(np.any(np.asarray(inputs[k], np.float32))
               for k in ("bk", "bv", "bo"))


def _make_in_maps(inputs, has_bias):
    bf16 = ml_dtypes.bfloat16
    x = np.asarray(inputs["x"], dtype=np.float32)
    ws = {k: np.ascontiguousarray(np.asarray(inputs[k], np.float32).astype(bf16))
          for k in ("Wq", "Wk", "Wv", "Wo")}
    bq = np.asarray(inputs["bq"], np.float32)
    bqc = np.ascontiguousarray(bq.reshape(KC, P).T.astype(np.float32))
    brow = {k: np.ascontiguousarray(
                np.asarray(inputs[k], np.float32).astype(bf16).reshape(1, DM))
            for k in ("bk", "bv", "bo")}
    xb = x.astype(bf16)
    in_maps = []
    for c in range(N_CORES):
        xs = np.ascontiguousarray(
            xb[:, c * SC:(c + 1) * SC, :].reshape(R, DM).T)
        m = {
            "x": xs,
            "wq": ws["Wq"], "wk": ws["Wk"], "wv": ws["Wv"], "wo": ws["Wo"],
            "bqc": bqc,
        }
        if has_bias:
            m.update({"bk2": brow["bk"], "bv2": brow["bv"],
                      "bo2": brow["bo"]})
        in_maps.append(m)
    return in_maps


def _run(inputs, **kw):
    from concourse import bass_utils
    hb = _has_bias(inputs)
    nc = _get_nc(hb)
    in_maps = _make_in_maps(inputs, hb)
    res = bass_utils.run_bass_kernel_spmd(
        nc, in_maps, core_ids=list(range(N_CORES)), **kw)
    out = np.empty((B, S, DM), np.float32)
    for c in range(N_CORES):
        out[:, c * SC:(c + 1) * SC, :] = res.results[c]["out"].reshape(B, SC, DM)
    return out, res


def kernel(**inputs) -> np.ndarray:
    out, _ = _run(inputs)
    return out
